# revision 17
# baseline (speedup 1.0000x reference)
import os
import sys

for _p in ("/opt/trn_rl_repo", "/root/.axon_site/_ro/trn_rl_repo"):
    if os.path.isdir(_p) and _p not in sys.path:
        sys.path.insert(0, _p)

import numpy as np

C, H, W = 8, 2048, 2048
NSEG = 64
NCORES = 8
P = 128
ROWS_PER_CORE = H // NCORES          # 256
SH = ROWS_PER_CORE * W               # 524288 pixels per core
F = SH // P                          # 4096 free elements per partition
T = 1024                             # free-dim tile per pass
NPASS = F // T
NV = 18                              # value cols: 8 pred, 8 pred*rmask, kmask, ones
GRP = 32                             # one-hot chunks built per DVE instruction
SIGMA_AGG = 0.5


_CACHE = {}


def _build_bass():
    import concourse.bacc as bacc
    import concourse.mybir as mybir
    from concourse.tile import TileContext

    fp32 = mybir.dt.float32
    bf16 = mybir.dt.bfloat16
    i32 = mybir.dt.int32
    Alu = mybir.AluOpType
    Act = mybir.ActivationFunctionType

    nc = bacc.Bacc("TRN2", target_bir_lowering=False, debug=False)

    pred_d = nc.dram_tensor("pred", [C, P, F], fp32, kind="ExternalInput")
    kl_d = nc.dram_tensor("kl", [P, F], i32, kind="ExternalInput")
    rl_d = nc.dram_tensor("rl", [P, F], i32, kind="ExternalInput")
    km_d = nc.dram_tensor("kmask", [P, F], fp32, kind="ExternalInput")
    rm_d = nc.dram_tensor("rmask", [P, F], fp32, kind="ExternalInput")

    seg_o = nc.dram_tensor("seg_out", [NV, NSEG], fp32, kind="ExternalOutput")
    s1_o = nc.dram_tensor("s1_out", [P, C * NPASS], fp32, kind="ExternalOutput")
    hist_o = nc.dram_tensor("hist_out", [P, NSEG * NPASS], fp32, kind="ExternalOutput")
    max_o = nc.dram_tensor("max_out", [P, 1], fp32, kind="ExternalOutput")
    pehist_o = nc.dram_tensor("pehist_out", [NSEG, 4], fp32, kind="ExternalOutput")

    with TileContext(nc) as tc:
        with (
            tc.tile_pool(name="const", bufs=1) as constp,
            tc.tile_pool(name="res", bufs=1) as resp,
            tc.tile_pool(name="vals", bufs=2) as valsp,
            tc.tile_pool(name="stage", bufs=3) as stagep,
            tc.tile_pool(name="small", bufs=2) as smallp,
            tc.tile_pool(name="oh", bufs=4) as ohp,
            tc.tile_pool(name="scr", bufs=2) as scrp,
            tc.tile_pool(name="hsc", bufs=6) as hscp,
            tc.tile_pool(name="psum", bufs=1, space="PSUM") as psump,
        ):
            # iota_rep[p, g*64 + s] = s  (fp32), for the broadcast is_equal
            iota_rep_i = constp.tile([P, GRP * NSEG], i32)
            nc.gpsimd.iota(iota_rep_i[:, :], pattern=[[0, GRP], [1, NSEG]],
                           base=0, channel_multiplier=0)
            iota_rep = constp.tile([P, GRP * NSEG], fp32)
            nc.vector.tensor_copy(iota_rep[:, :], iota_rep_i[:, :])
            iota_rep3 = iota_rep.rearrange("p (g s) -> p g s", g=GRP)
            # ZO[p, j] = 1 iff j == 63; windows ZO[:, 63-r : 127-r] select row r
            zo = constp.tile([P, 127], bf16)
            nc.vector.memset(zo[:, :], 0.0)
            nc.vector.memset(zo[:, 63:64], 1.0)

            rlb = resp.tile([P, F], bf16)      # region labels as bf16
            s1cols = resp.tile([P, C * NPASS], fp32)
            histc = resp.tile([P, NSEG * NPASS], fp32)
            maxc = resp.tile([P, 1], fp32)
            nc.vector.memset(histc[:, :], 0.0)

            psum_t = psump.tile([NV, NSEG], fp32, tag="main")
            psum_h = psump.tile([NSEG, T], fp32, tag="ph")

            for k in range(NPASS):
                sl = slice(k * T, (k + 1) * T)
                vals = valsp.tile([P, T * NV], bf16, tag="vals")
                vv = vals.rearrange("p (j t) -> p j t", j=NV)

                kl_t = stagep.tile([P, T], i32, tag="kl")
                rl_t = stagep.tile([P, T], i32, tag="rl")
                rm_t = stagep.tile([P, T], fp32, tag="rm")
                km_t = stagep.tile([P, T], fp32, tag="km")
                klb = smallp.tile([P, T], fp32, tag="klb")

                nc.sync.dma_start(kl_t[:, :], kl_d[:, sl])
                nc.sync.dma_start(rl_t[:, :], rl_d[:, sl])
                nc.sync.dma_start(rm_t[:, :], rm_d[:, sl])
                nc.sync.dma_start(km_t[:, :], km_d[:, sl])

                nc.scalar.copy(klb[:, :], kl_t[:, :])
                nc.scalar.copy(rlb[:, sl], rl_t[:, :])
                nc.scalar.copy(vv[:, 16, :], km_t[:, :])
                nc.vector.memset(vv[:, 17, :], 1.0)

                for c in range(C):
                    pr = stagep.tile([P, T], fp32, tag="pred")
                    nc.sync.dma_start(pr[:, :], pred_d[c, :, sl])
                    nc.scalar.copy(vv[:, c, :], pr[:, :])
                    nc.vector.tensor_tensor(
                        vv[:, 8 + c, :], pr[:, :], rm_t[:, :], op=Alu.mult
                    )
                    sq = scrp.tile([P, T], bf16, tag="sq")
                    nc.scalar.activation(
                        sq[:, :], vv[:, 8 + c, :], Act.Square,
                        accum_out=s1cols[:, k * C + c : k * C + c + 1],
                    )

                klb3 = klb.rearrange("p (t o) -> p t o", o=1)
                NGRP = T // GRP
                for g in range(NGRP):
                    oh8 = ohp.tile([P, GRP * NSEG], bf16, tag="oh")
                    oh83 = oh8.rearrange("p (g s) -> p g s", g=GRP)
                    lab_b = klb3[:, g * GRP : (g + 1) * GRP, :].broadcast_to(
                        (P, GRP, NSEG)
                    )
                    nc.vector.tensor_tensor(
                        oh83[:, :, :], lab_b, iota_rep3[:, :, :], op=Alu.is_equal
                    )
                    for tt in range(GRP):
                        t = g * GRP + tt
                        nc.tensor.matmul(
                            psum_t[:, :],
                            lhsT=vv[:, :, t],
                            rhs=oh8[:, tt * NSEG : (tt + 1) * NSEG],
                            start=(k == 0 and t == 0),
                            stop=(k == NPASS - 1 and t == T - 1),
                        )
                    # interleaved histogram scans (NSEG/NGRP labels per group)
                    LPG = NSEG // NGRP
                    for r in range(g * LPG, (g + 1) * LPG):
                        hcol = histc[:, k * NSEG + r : k * NSEG + r + 1]
                        h1 = hscp.tile([P, T], bf16, tag="hsc")
                        nc.vector.tensor_scalar(
                            h1[:, :], rlb[:, sl], float(r), None,
                            op0=Alu.is_equal,
                        )
                        if r % 2 == 0:
                            h1s = scrp.tile([P, T], bf16, tag="hsq")
                            nc.scalar.activation(
                                h1s[:, :], h1[:, :], Act.Square, accum_out=hcol,
                            )
                        else:
                            for b in range(T // 512):
                                bs = slice(b * 512, (b + 1) * 512)
                                nc.tensor.matmul(
                                    psum_h[:, bs], lhsT=zo[:, 63 - r : 127 - r],
                                    rhs=h1[:, bs],
                                    start=(k == 0 and r == 1),
                                    stop=(k == NPASS - 1 and r == NSEG - 1),
                                )

            nc.vector.tensor_reduce(
                maxc[:, :], rlb[:, :], axis=mybir.AxisListType.X, op=Alu.max
            )

            pefin = resp.tile([NSEG, 4], fp32)
            nc.vector.memset(pefin[:, :], 0.0)
            pescr = resp.tile([NSEG, T], bf16)
            for b in range(T // 512):
                bs = slice(b * 512, (b + 1) * 512)
                nc.scalar.activation(
                    pescr[:, bs], psum_h[:, bs], Act.Identity,
                    accum_out=pefin[:, b : b + 1],
                )
            nc.sync.dma_start(pehist_o[:, :], pefin[:, :])
            seg_sb = resp.tile([NV, NSEG], fp32)
            nc.vector.tensor_copy(seg_sb[:, :], psum_t[:, :])
            nc.sync.dma_start(seg_o[:, :], seg_sb[:, :])
            nc.sync.dma_start(s1_o[:, :], s1cols[:, :])
            nc.sync.dma_start(hist_o[:, :], histc[:, :])
            nc.sync.dma_start(max_o[:, :], maxc[:, :])

    nc.compile()
    return nc


def _get_nc():
    if "nc" not in _CACHE:
        _CACHE["nc"] = _build_bass()
    return _CACHE["nc"]


def _numpy_fallback(pred, rmask, kmask, kl, rl):
    klf = kl.reshape(-1)
    rlf = rl.reshape(-1)
    kcard = np.zeros(NSEG, np.float64)
    np.add.at(kcard, klf, kmask.reshape(-1).astype(np.float64))
    rcard = np.zeros(NSEG, np.float64)
    np.add.at(rcard, rlf, rmask.reshape(-1).astype(np.float64))
    predf = pred.reshape(C, -1).astype(np.float64)
    seg = np.zeros((C, NSEG), np.float64)
    for c in range(C):
        np.add.at(seg[c], klf, predf[c])
    g = np.where(np.arange(NSEG)[None, :] > 0, seg, 0.0) / (kcard + 1.0)[None, :]
    Fp = predf * rmask.reshape(-1)[None, :].astype(np.float64)
    diff = Fp - g[:, klf]
    D = max(np.sqrt(np.sum(diff * diff)) - SIGMA_AGG, 0.0)
    L = np.log(D * D + 1.0)
    pixsum = np.sum(1.0 / (rcard[rlf] + 1.0))
    num_region = max(rl.max(), 1)
    return np.float32(L * pixsum / num_region)


def kernel(**inputs):
    from concourse import bass_utils

    pred = np.asarray(inputs["pred_similarities"], dtype=np.float32)
    rmask = np.asarray(inputs["regions_mask"], dtype=np.float32)
    kmask = np.asarray(inputs["kernels_mask"], dtype=np.float32)
    kl = np.asarray(inputs["kernel_labels"], dtype=np.int32)
    rl = np.asarray(inputs["region_labels"], dtype=np.int32)

    # the fast path derives rcard from hist_r, valid iff regions_mask is
    # exactly the indicator of region_labels > 0; verify and bail otherwise
    if not np.array_equal(rmask, (rl > 0).astype(np.float32)):
        return _numpy_fallback(pred, rmask, kmask, kl, rl)

    nc = _get_nc()

    in_maps = []
    for ci in range(NCORES):
        rows = slice(ci * ROWS_PER_CORE, (ci + 1) * ROWS_PER_CORE)
        in_maps.append({
            "pred": np.ascontiguousarray(pred[:, rows, :]).reshape(C, P, F),
            "kl": np.ascontiguousarray(kl[rows, :]).reshape(P, F),
            "rl": np.ascontiguousarray(rl[rows, :]).reshape(P, F),
            "kmask": np.ascontiguousarray(kmask[rows, :]).reshape(P, F),
            "rmask": np.ascontiguousarray(rmask[rows, :]).reshape(P, F),
        })

    res = bass_utils.run_bass_kernel_spmd(nc, in_maps, core_ids=list(range(NCORES)))

    seg = np.zeros((NV, NSEG), np.float64)
    s1 = 0.0
    hist_r = np.zeros(NSEG, np.float64)
    rcard = np.zeros(NSEG, np.float64)
    maxrl = 0.0
    for r in res.results:
        seg += r["seg_out"].astype(np.float64)
        s1 += r["s1_out"].astype(np.float64).sum()
        hist_r += (
            r["hist_out"].astype(np.float64).sum(axis=0)
            .reshape(NPASS, NSEG).sum(axis=0)
        )
        pe = r["pehist_out"].astype(np.float64)
        hist_r += pe[:, 0] + pe[:, 1]
        maxrl = max(maxrl, r["max_out"].max())

    B = seg[0:C, :]            # [C, NSEG] sum of pred per kernel label
    A = seg[C:2 * C, :]        # [C, NSEG] sum of pred*rmask per kernel label
    kcard = seg[16, :]
    hist_k = seg[17, :]

    mask_s = (np.arange(NSEG) > 0).astype(np.float64)
    g = mask_s[None, :] * B / (kcard + 1.0)[None, :]

    sumsq = s1 - 2.0 * np.sum(A * g) + np.sum(hist_k[None, :] * g * g)
    D = max(np.sqrt(max(sumsq, 0.0)) - SIGMA_AGG, 0.0)
    L = np.log(D * D + 1.0)
    rcard = hist_r.copy()
    rcard[0] = 0.0
    pixsum = np.sum(hist_r / (rcard + 1.0))
    num_region = max(float(maxrl), 1.0)
    return np.float32(L * pixsum / num_region)
